# revision 6
# baseline (speedup 1.0000x reference)
"""MoE layer (dense routing) Trainium2 kernel.

Problem: B=4, S=2048, H=1024, I=2816, E=8 experts, every expert runs on every
token weighted by full softmax routing weights.

Sharding: data-parallel over tokens. Each of the 8 NeuronCores processes 1024
tokens through all 8 experts (identical NEFF, different input slices).

Per-core dataflow (all "T" layouts keep tokens on the free axis, so the
contraction dim of every matmul sits on SBUF partitions):
  xT   [128p, 8kc, 1024t]  fp32r   X^T for this core's tokens (H on partitions)
  gate psum[i,t] = sum_k  G[e,i][128k,128i].T @ xT[k, t*512:+512]   (fp32r)
  up   likewise                                                     (fp32r)
  P'   [128p(i), 22ic, 1024t] fp16  = silu(gate) * up
  down psum[h,t] = sum_i  D[e,h][128i,128h].T @ P'[i, t*512:+512]   (fp16)
  out_acc [128p(h), 8hc, 1024t] fp32 += down_psum * w_bcast[e]
Routing weights (softmax over a tiny [T,8] router matmul — 0.01% of the FLOPs)
and the balance loss are computed on the host in fp32; the per-expert weights
are shipped pre-broadcast as [8, 128, 1024] so the device multiply is a plain
tensor_tensor.
"""

import numpy as np

B, S, H, I, E = 4, 2048, 1024, 2816, 8
BALANCE_COEF = 0.01
NCORES = 8
P = 128
TC = (B * S) // NCORES  # tokens per core = 1024
TSUB = 512  # psum free-dim slice
NT = TC // TSUB  # 2
NK = H // P  # 8 contraction chunks for gate/up
NI = I // P  # 22 i-tiles
NH = H // P  # 8 output h-tiles

_cache = {}


def _build_nc():
    import concourse.mybir as mybir
    import concourse.tile as tile
    from concourse import bacc

    f32 = mybir.dt.float32
    f32r = mybir.dt.float32r
    f16 = mybir.dt.float16

    nc = bacc.Bacc("TRN2", target_bir_lowering=False, debug=False, num_devices=NCORES)

    xr = nc.dram_tensor("xr", [P, NK, TC], f32r, kind="ExternalInput").ap()
    gw = nc.dram_tensor("gw", [E, NI, P, NK, P], f32r, kind="ExternalInput").ap()
    uw = nc.dram_tensor("uw", [E, NI, P, NK, P], f32r, kind="ExternalInput").ap()
    dw = nc.dram_tensor("dw", [E, NH, P, NI, P], f16, kind="ExternalInput").ap()
    wb = nc.dram_tensor("wb", [E, P, TC], f32, kind="ExternalInput").ap()
    oT = nc.dram_tensor("oT", [P, NH, TC], f32, kind="ExternalOutput").ap()

    with tile.TileContext(nc) as tc:
        with (
            tc.tile_pool(name="resident", bufs=1) as resident,
            tc.tile_pool(name="gup", bufs=3) as gup,
            tc.tile_pool(name="dwp", bufs=2) as dwp,
            tc.tile_pool(name="wbp", bufs=2) as wbp,
            tc.tile_pool(name="pprime", bufs=1) as pprime,
            tc.tile_pool(name="tmp", bufs=4) as tmp,
            tc.tile_pool(name="gps", bufs=1, space="PSUM") as gps,
            tc.tile_pool(name="ups", bufs=1, space="PSUM") as ups,
            tc.tile_pool(name="dps", bufs=2, space="PSUM") as dps,
        ):
            xr_sb = resident.tile([P, NK, TC], f32r, name="xr_sb")
            for k in range(NK):
                nc.sync.dma_start(xr_sb[:, k, :], xr[:, k, :])
            out_acc = resident.tile([P, NH, TC], f32, name="out_acc")

            for e in range(E):
                wb_sb = wbp.tile([P, TC], f32, tag="wb", name="wb_sb")
                nc.sync.dma_start(wb_sb[:], wb[e])

                pp = pprime.tile([P, NI, TC], f16, tag="pp", name="pp")

                # ---- gate / up ----
                # k outer / t inner: each 128x128 weight chunk is the lhsT for
                # both token halves, halving PE weight loads.
                for i in range(NI):
                    g_sb = gup.tile([P, NK, P], f32r, tag="g", name="g_sb")
                    nc.sync.dma_start(g_sb[:], gw[e, i])
                    u_sb = gup.tile([P, NK, P], f32r, tag="u", name="u_sb")
                    nc.sync.dma_start(u_sb[:], uw[e, i])
                    gp = gps.tile([P, NT, TSUB], mybir.dt.float32, tag="gp", name="gp")
                    for k in range(NK):
                        for t in range(NT):
                            nc.tensor.matmul(
                                gp[:, t, :],
                                g_sb[:, k, :],
                                xr_sb[:, k, t * TSUB : (t + 1) * TSUB],
                                start=(k == 0),
                                stop=(k == NK - 1),
                            )
                    up = ups.tile([P, NT, TSUB], mybir.dt.float32, tag="up", name="up")
                    for k in range(NK):
                        for t in range(NT):
                            nc.tensor.matmul(
                                up[:, t, :],
                                u_sb[:, k, :],
                                xr_sb[:, k, t * TSUB : (t + 1) * TSUB],
                                start=(k == 0),
                                stop=(k == NK - 1),
                            )
                    sl = tmp.tile([P, NT, TSUB], mybir.dt.float32, tag="sl", name="sl")
                    nc.scalar.activation(
                        sl[:], gp[:], mybir.ActivationFunctionType.Silu
                    )
                    nc.vector.tensor_tensor(
                        pp[:, i, :], sl[:].rearrange("p t n -> p (t n)"), up[:].rearrange("p t n -> p (t n)"), mybir.AluOpType.mult
                    )

                # ---- down + weighted accumulate ----
                for h in range(NH):
                    d_sb = dwp.tile([P, NI, P], f16, tag="d", name="d_sb")
                    nc.sync.dma_start(d_sb[:], dw[e, h])
                    for t in range(NT):
                        ts = slice(t * TSUB, (t + 1) * TSUB)
                        dp = dps.tile([P, TSUB], mybir.dt.float32, tag="dp", name="dp")
                        for i in range(NI):
                            nc.tensor.matmul(
                                dp[:],
                                d_sb[:, i, :],
                                pp[:, i, ts],
                                start=(i == 0),
                                stop=(i == NI - 1),
                            )
                        if e == 0:
                            nc.vector.tensor_tensor(
                                out_acc[:, h, ts],
                                dp[:],
                                wb_sb[:, ts],
                                mybir.AluOpType.mult,
                            )
                        else:
                            wt = tmp.tile(
                                [P, TSUB], mybir.dt.float32, tag="wt", name="wt"
                            )
                            nc.vector.tensor_tensor(
                                wt[:], dp[:], wb_sb[:, ts], mybir.AluOpType.mult
                            )
                            nc.vector.tensor_add(
                                out_acc[:, h, ts], out_acc[:, h, ts], wt[:]
                            )
                    if e == E - 1:
                        # stream each finished h-slice out during the last expert
                        nc.sync.dma_start(oT[:, h, :], out_acc[:, h, :])
    nc.compile()
    return nc


def _get_nc():
    if "nc" not in _cache:
        _cache["nc"] = _build_nc()
    return _cache["nc"]


def _prep_shared(gate_w, up_w, down_w):
    """Host-side swizzle of the stacked expert weights into the DMA-friendly
    pre-tiled layouts (same arrays shared by all cores)."""
    gw = np.ascontiguousarray(
        gate_w.reshape(E, NK, P, NI, P).transpose(0, 3, 2, 1, 4), dtype=np.float32
    )
    uw = np.ascontiguousarray(
        up_w.reshape(E, NK, P, NI, P).transpose(0, 3, 2, 1, 4), dtype=np.float32
    )
    dw = (
        down_w.reshape(E, NI, P, NH, P)
        .transpose(0, 3, 2, 1, 4)
        .astype(np.float16)
    )
    dw = np.ascontiguousarray(dw)
    return gw, uw, dw


def kernel(hidden_states, router_w, router_b, gate_w, up_w, down_w):
    from concourse.bass_utils import run_bass_kernel_spmd

    hidden_states = np.asarray(hidden_states, dtype=np.float32)
    router_w = np.asarray(router_w, dtype=np.float32)
    router_b = np.asarray(router_b, dtype=np.float32)
    gate_w = np.asarray(gate_w, dtype=np.float32)
    up_w = np.asarray(up_w, dtype=np.float32)
    down_w = np.asarray(down_w, dtype=np.float32)

    X = hidden_states.reshape(B * S, H)

    # ---- host routing (tiny: [8192,1024]x[1024,8]) + balance loss, fp32 ----
    logits = X @ router_w.T + router_b
    logits = logits - logits.max(axis=-1, keepdims=True)
    ex = np.exp(logits)
    rw = ex / ex.sum(axis=-1, keepdims=True)  # [B*S, E] fp32
    mean_routing = rw.reshape(B, S, E).mean(axis=0)
    balance_loss = np.float32(
        np.mean((mean_routing - np.float32(1.0 / E)) ** 2) * np.float32(BALANCE_COEF)
    )

    gw, uw, dw = _prep_shared(gate_w, up_w, down_w)

    in_maps = []
    for c in range(NCORES):
        tok = slice(c * TC, (c + 1) * TC)
        xr = np.ascontiguousarray(
            X[tok].T.reshape(NK, P, TC).transpose(1, 0, 2)
        )  # [P, NK, TC] fp32
        rw_c = rw[tok].T  # [E, TC]
        wb = np.ascontiguousarray(
            np.broadcast_to(rw_c[:, None, :], (E, P, TC)), dtype=np.float32
        )
        in_maps.append({"xr": xr, "gw": gw, "uw": uw, "dw": dw, "wb": wb})

    nc = _get_nc()
    import os

    trace = os.environ.get("MOE_TRACE", "0") == "1"
    res = run_bass_kernel_spmd(
        nc, in_maps, core_ids=list(range(NCORES)), trace=trace
    )
    _cache["last_result"] = res

    out = np.empty((B * S, H), dtype=np.float32)
    for c in range(NCORES):
        oT = res.results[c]["oT"]  # [P, NH, TC]
        out[c * TC : (c + 1) * TC] = oT.transpose(1, 0, 2).reshape(H, TC).T
    out = out.reshape(B, S, H)
    return out, balance_loss


# revision 8
# speedup vs baseline: 1.0696x; 1.0696x over previous
"""MoE layer (dense routing) Trainium2 kernel.

Problem: B=4, S=2048, H=1024, I=2816, E=8 experts, every expert runs on every
token weighted by full softmax routing weights.

Sharding: data-parallel over tokens. Each of the 8 NeuronCores processes 1024
tokens through all 8 experts (identical NEFF, different input slices).

Per-core dataflow (all "T" layouts keep tokens on the free axis, so the
contraction dim of every matmul sits on SBUF partitions):
  xT   [128p, 8kc, 1024t]  fp32r   X^T for this core's tokens (H on partitions)
  gate psum[i,t] = sum_k  G[e,i][128k,128i].T @ xT[k, t*512:+512]   (fp32r)
  up   likewise                                                     (fp32r)
  P'   [128p(i), 22ic, 1024t] fp16  = silu(gate) * up
  down psum[h,t] = sum_i  D[e,h][128i,128h].T @ P'[i, t*512:+512]   (fp16)
  out_acc [128p(h), 8hc, 1024t] fp32 += down_psum * w_bcast[e]
Routing weights (softmax over a tiny [T,8] router matmul — 0.01% of the FLOPs)
and the balance loss are computed on the host in fp32; the per-expert weights
are shipped pre-broadcast as [8, 128, 1024] so the device multiply is a plain
tensor_tensor.
"""

import numpy as np

B, S, H, I, E = 4, 2048, 1024, 2816, 8
BALANCE_COEF = 0.01
NCORES = 8
P = 128
TC = (B * S) // NCORES  # tokens per core = 1024
TSUB = 512  # psum free-dim slice
NT = TC // TSUB  # 2
NK = H // P  # 8 contraction chunks for gate/up
NI = I // P  # 22 i-tiles
NH = H // P  # 8 output h-tiles

_cache = {}


def _build_nc():
    import concourse.mybir as mybir
    import concourse.tile as tile
    from concourse import bacc

    f32 = mybir.dt.float32
    f32r = mybir.dt.float32r
    f16 = mybir.dt.float16

    nc = bacc.Bacc("TRN2", target_bir_lowering=False, debug=False, num_devices=NCORES)

    xr = nc.dram_tensor("xr", [P, NK, TC], f32r, kind="ExternalInput").ap()
    gw = nc.dram_tensor("gw", [E, NI, P, NK, P], f32r, kind="ExternalInput").ap()
    uw = nc.dram_tensor("uw", [E, NI, P, NK, P], f32r, kind="ExternalInput").ap()
    dw = nc.dram_tensor("dw", [E, NH, P, NI, P], f16, kind="ExternalInput").ap()
    wb = nc.dram_tensor("wb", [E, P, TC], f32, kind="ExternalInput").ap()
    oT = nc.dram_tensor("oT", [P, NH, TC], f32, kind="ExternalOutput").ap()

    with tile.TileContext(nc) as tc:
        with (
            tc.tile_pool(name="resident", bufs=1) as resident,
            tc.tile_pool(name="gup", bufs=3) as gup,
            tc.tile_pool(name="dwp", bufs=2) as dwp,
            tc.tile_pool(name="wbp", bufs=2) as wbp,
            tc.tile_pool(name="pprime", bufs=1) as pprime,
            tc.tile_pool(name="tmp", bufs=4) as tmp,
            tc.tile_pool(name="gps", bufs=1, space="PSUM") as gps,
            tc.tile_pool(name="ups", bufs=1, space="PSUM") as ups,
            tc.tile_pool(name="dps", bufs=2, space="PSUM") as dps,
        ):
            # xr as per-k tiles so the first matmuls only wait on chunk k=0
            xr_sb = []
            xr_first = resident.tile([P, TC], f32r, name="xr_sb0", tag="xr0")
            for k in range(NK):
                if k == 0:
                    xr_sb.append(xr_first)
                else:
                    xr_sb.append(resident.tile([P, TC], f32r, name=f"xr_sb{k}", tag=f"xr{k}"))
            out_acc = resident.tile([P, NH, TC], f32, name="out_acc")

            for e in range(E):
                wb_sb = wbp.tile([P, TC], f32, tag="wb", name="wb_sb")
                nc.sync.dma_start(wb_sb[:], wb[e])

                pp = pprime.tile([P, NI, TC], f16, tag="pp", name="pp")

                # ---- gate / up ----
                # k outer / t inner: each 128x128 weight chunk is the lhsT for
                # both token halves, halving PE weight loads.
                for i in range(NI):
                    g_sb = gup.tile([P, NK, P], f32r, tag="g", name="g_sb")
                    nc.sync.dma_start(g_sb[:], gw[e, i])
                    u_sb = gup.tile([P, NK, P], f32r, tag="u", name="u_sb")
                    nc.sync.dma_start(u_sb[:], uw[e, i])
                    if e == 0 and i == 0:
                        # stage the activations after the first weight tiles
                        for k in range(NK):
                            nc.sync.dma_start(xr_sb[k][:], xr[:, k, :])
                    gp = gps.tile([P, NT, TSUB], mybir.dt.float32, tag="gp", name="gp")
                    for k in range(NK):
                        for t in range(NT):
                            nc.tensor.matmul(
                                gp[:, t, :],
                                g_sb[:, k, :],
                                xr_sb[k][:, t * TSUB : (t + 1) * TSUB],
                                start=(k == 0),
                                stop=(k == NK - 1),
                            )
                    up = ups.tile([P, NT, TSUB], mybir.dt.float32, tag="up", name="up")
                    for k in range(NK):
                        for t in range(NT):
                            nc.tensor.matmul(
                                up[:, t, :],
                                u_sb[:, k, :],
                                xr_sb[k][:, t * TSUB : (t + 1) * TSUB],
                                start=(k == 0),
                                stop=(k == NK - 1),
                            )
                    sl = tmp.tile([P, NT, TSUB], mybir.dt.float32, tag="sl", name="sl")
                    nc.scalar.activation(
                        sl[:], gp[:], mybir.ActivationFunctionType.Silu
                    )
                    nc.vector.tensor_tensor(
                        pp[:, i, :], sl[:].rearrange("p t n -> p (t n)"), up[:].rearrange("p t n -> p (t n)"), mybir.AluOpType.mult
                    )

                # ---- down + weighted accumulate ----
                for h in range(NH):
                    d_sb = dwp.tile([P, NI, P], f16, tag="d", name="d_sb")
                    nc.sync.dma_start(d_sb[:], dw[e, h])
                    for t in range(NT):
                        ts = slice(t * TSUB, (t + 1) * TSUB)
                        dp = dps.tile([P, TSUB], mybir.dt.float32, tag="dp", name="dp")
                        for i in range(NI):
                            nc.tensor.matmul(
                                dp[:],
                                d_sb[:, i, :],
                                pp[:, i, ts],
                                start=(i == 0),
                                stop=(i == NI - 1),
                            )
                        if e == 0:
                            nc.vector.tensor_tensor(
                                out_acc[:, h, ts],
                                dp[:],
                                wb_sb[:, ts],
                                mybir.AluOpType.mult,
                            )
                        else:
                            wt = tmp.tile(
                                [P, TSUB], mybir.dt.float32, tag="wt", name="wt"
                            )
                            nc.vector.tensor_tensor(
                                wt[:], dp[:], wb_sb[:, ts], mybir.AluOpType.mult
                            )
                            nc.vector.tensor_add(
                                out_acc[:, h, ts], out_acc[:, h, ts], wt[:]
                            )
                    if e == E - 1:
                        # stream each finished h-slice out during the last expert
                        nc.sync.dma_start(oT[:, h, :], out_acc[:, h, :])
    nc.compile()
    return nc


def _get_nc():
    if "nc" not in _cache:
        _cache["nc"] = _build_nc()
    return _cache["nc"]


def _prep_shared(gate_w, up_w, down_w):
    """Host-side swizzle of the stacked expert weights into the DMA-friendly
    pre-tiled layouts (same arrays shared by all cores)."""
    gw = np.ascontiguousarray(
        gate_w.reshape(E, NK, P, NI, P).transpose(0, 3, 2, 1, 4), dtype=np.float32
    )
    uw = np.ascontiguousarray(
        up_w.reshape(E, NK, P, NI, P).transpose(0, 3, 2, 1, 4), dtype=np.float32
    )
    dw = (
        down_w.reshape(E, NI, P, NH, P)
        .transpose(0, 3, 2, 1, 4)
        .astype(np.float16)
    )
    dw = np.ascontiguousarray(dw)
    return gw, uw, dw


def kernel(hidden_states, router_w, router_b, gate_w, up_w, down_w):
    from concourse.bass_utils import run_bass_kernel_spmd

    hidden_states = np.asarray(hidden_states, dtype=np.float32)
    router_w = np.asarray(router_w, dtype=np.float32)
    router_b = np.asarray(router_b, dtype=np.float32)
    gate_w = np.asarray(gate_w, dtype=np.float32)
    up_w = np.asarray(up_w, dtype=np.float32)
    down_w = np.asarray(down_w, dtype=np.float32)

    X = hidden_states.reshape(B * S, H)

    # ---- host routing (tiny: [8192,1024]x[1024,8]) + balance loss, fp32 ----
    logits = X @ router_w.T + router_b
    logits = logits - logits.max(axis=-1, keepdims=True)
    ex = np.exp(logits)
    rw = ex / ex.sum(axis=-1, keepdims=True)  # [B*S, E] fp32
    mean_routing = rw.reshape(B, S, E).mean(axis=0)
    balance_loss = np.float32(
        np.mean((mean_routing - np.float32(1.0 / E)) ** 2) * np.float32(BALANCE_COEF)
    )

    gw, uw, dw = _prep_shared(gate_w, up_w, down_w)

    in_maps = []
    for c in range(NCORES):
        tok = slice(c * TC, (c + 1) * TC)
        xr = np.ascontiguousarray(
            X[tok].T.reshape(NK, P, TC).transpose(1, 0, 2)
        )  # [P, NK, TC] fp32
        rw_c = rw[tok].T  # [E, TC]
        wb = np.ascontiguousarray(
            np.broadcast_to(rw_c[:, None, :], (E, P, TC)), dtype=np.float32
        )
        in_maps.append({"xr": xr, "gw": gw, "uw": uw, "dw": dw, "wb": wb})

    nc = _get_nc()
    import os

    trace = os.environ.get("MOE_TRACE", "0") == "1"
    res = run_bass_kernel_spmd(
        nc, in_maps, core_ids=list(range(NCORES)), trace=trace
    )
    _cache["last_result"] = res

    out = np.empty((B * S, H), dtype=np.float32)
    for c in range(NCORES):
        oT = res.results[c]["oT"]  # [P, NH, TC]
        out[c * TC : (c + 1) * TC] = oT.transpose(1, 0, 2).reshape(H, TC).T
    out = out.reshape(B, S, H)
    return out, balance_loss


# revision 10
# speedup vs baseline: 1.0707x; 1.0010x over previous
"""MoE layer (dense routing) Trainium2 kernel.

Problem: B=4, S=2048, H=1024, I=2816, E=8 experts, every expert runs on every
token weighted by full softmax routing weights.

Sharding: data-parallel over tokens. Each of the 8 NeuronCores processes 1024
tokens through all 8 experts (identical NEFF, different input slices).

Per-core dataflow (all "T" layouts keep tokens on the free axis, so the
contraction dim of every matmul sits on SBUF partitions):
  xT   [128p, 8kc, 1024t]  fp32r   X^T for this core's tokens (H on partitions)
  gate psum[i,t] = sum_k  G[e,i][128k,128i].T @ xT[k, t*512:+512]   (fp32r)
  up   likewise                                                     (fp32r)
  P'   [128p(i), 22ic, 1024t] fp16  = silu(gate) * up
  down psum[h,t] = sum_i  D[e,h][128i,128h].T @ P'[i, t*512:+512]   (fp16)
  out_acc [128p(h), 8hc, 1024t] fp32 += down_psum * w_bcast[e]
Routing weights (softmax over a tiny [T,8] router matmul — 0.01% of the FLOPs)
and the balance loss are computed on the host in fp32; the per-expert weights
are shipped pre-broadcast as [8, 128, 1024] so the device multiply is a plain
tensor_tensor.
"""

import numpy as np

B, S, H, I, E = 4, 2048, 1024, 2816, 8
BALANCE_COEF = 0.01
NCORES = 8
P = 128
TC = (B * S) // NCORES  # tokens per core = 1024
TSUB = 512  # psum free-dim slice
NT = TC // TSUB  # 2
NK = H // P  # 8 contraction chunks for gate/up
NI = I // P  # 22 i-tiles
NH = H // P  # 8 output h-tiles

_cache = {}


def _build_nc():
    import concourse.mybir as mybir
    import concourse.tile as tile
    from concourse import bacc

    f32 = mybir.dt.float32
    f32r = mybir.dt.float32r
    f16 = mybir.dt.float16

    nc = bacc.Bacc("TRN2", target_bir_lowering=False, debug=False, num_devices=NCORES)

    xr = nc.dram_tensor("xr", [P, NK, TC], f32r, kind="ExternalInput").ap()
    gw = nc.dram_tensor("gw", [E, NI, P, NK, P], f32r, kind="ExternalInput").ap()
    uw = nc.dram_tensor("uw", [E, NI, P, NK, P], f32r, kind="ExternalInput").ap()
    dw = nc.dram_tensor("dw", [E, NH, P, NI, P], f16, kind="ExternalInput").ap()
    wb = nc.dram_tensor("wb", [E, P, TC], f32, kind="ExternalInput").ap()
    oT = nc.dram_tensor("oT", [P, NH, TC], f32, kind="ExternalOutput").ap()

    with tile.TileContext(nc) as tc:
        with (
            tc.tile_pool(name="resident", bufs=1) as resident,
            tc.tile_pool(name="gup", bufs=3) as gup,
            tc.tile_pool(name="dwp", bufs=2) as dwp,
            tc.tile_pool(name="wbp", bufs=2) as wbp,
            tc.tile_pool(name="pprime", bufs=1) as pprime,
            tc.tile_pool(name="tmp", bufs=4) as tmp,
            tc.tile_pool(name="gps", bufs=1, space="PSUM") as gps,
            tc.tile_pool(name="ups", bufs=1, space="PSUM") as ups,
            tc.tile_pool(name="dps", bufs=2, space="PSUM") as dps,
        ):
            # xr as per-k tiles so the first matmuls only wait on chunk k=0
            xr_sb = []
            xr_first = resident.tile([P, TC], f32r, name="xr_sb0", tag="xr0")
            for k in range(NK):
                if k == 0:
                    xr_sb.append(xr_first)
                else:
                    xr_sb.append(resident.tile([P, TC], f32r, name=f"xr_sb{k}", tag=f"xr{k}"))
            out_acc = resident.tile([P, NH, TC], f32, name="out_acc")

            for e in range(E):
                wb_sb = wbp.tile([P, TC], f32, tag="wb", name="wb_sb")
                nc.sync.dma_start(wb_sb[:], wb[e])

                pp = pprime.tile([P, NI, TC], f16, tag="pp", name="pp")

                # ---- gate / up ----
                # k outer / t inner: each 128x128 weight chunk is the lhsT for
                # both token halves, halving PE weight loads.
                for i in range(NI):
                    g_sb = gup.tile([P, NK, P], f32r, tag="g", name="g_sb")
                    nc.sync.dma_start(g_sb[:], gw[e, i])
                    u_sb = gup.tile([P, NK, P], f32r, tag="u", name="u_sb")
                    nc.sync.dma_start(u_sb[:], uw[e, i])
                    if e == 0 and i == 0:
                        # stage the activations after the first weight tiles
                        for k in range(NK):
                            nc.sync.dma_start(xr_sb[k][:], xr[:, k, :])
                    gp = gps.tile([P, NT, TSUB], mybir.dt.float32, tag="gp", name="gp")
                    for k in range(NK):
                        for t in range(NT):
                            nc.tensor.matmul(
                                gp[:, t, :],
                                g_sb[:, k, :],
                                xr_sb[k][:, t * TSUB : (t + 1) * TSUB],
                                start=(k == 0),
                                stop=(k == NK - 1),
                            )
                    up = ups.tile([P, NT, TSUB], mybir.dt.float32, tag="up", name="up")
                    for k in range(NK):
                        for t in range(NT):
                            nc.tensor.matmul(
                                up[:, t, :],
                                u_sb[:, k, :],
                                xr_sb[k][:, t * TSUB : (t + 1) * TSUB],
                                start=(k == 0),
                                stop=(k == NK - 1),
                            )
                    sl = tmp.tile([P, NT, TSUB], mybir.dt.float32, tag="sl", name="sl")
                    nc.scalar.activation(
                        sl[:], gp[:], mybir.ActivationFunctionType.Silu
                    )
                    nc.vector.tensor_tensor(
                        pp[:, i, :], sl[:].rearrange("p t n -> p (t n)"), up[:].rearrange("p t n -> p (t n)"), mybir.AluOpType.mult
                    )

                # ---- down + weighted accumulate ----
                for h in range(NH):
                    d_sb = dwp.tile([P, NI, P], f16, tag="d", name="d_sb")
                    nc.sync.dma_start(d_sb[:], dw[e, h])
                    for t in range(NT):
                        ts = slice(t * TSUB, (t + 1) * TSUB)
                        dp = dps.tile([P, TSUB], mybir.dt.float32, tag="dp", name="dp")
                        for i in range(NI):
                            nc.tensor.matmul(
                                dp[:],
                                d_sb[:, i, :],
                                pp[:, i, ts],
                                start=(i == 0),
                                stop=(i == NI - 1),
                            )
                        if e == 0:
                            nc.vector.tensor_tensor(
                                out_acc[:, h, ts],
                                dp[:],
                                wb_sb[:, ts],
                                mybir.AluOpType.mult,
                            )
                        else:
                            wt = tmp.tile(
                                [P, TSUB], mybir.dt.float32, tag="wt", name="wt"
                            )
                            nc.vector.tensor_tensor(
                                wt[:], dp[:], wb_sb[:, ts], mybir.AluOpType.mult
                            )
                            nc.vector.tensor_add(
                                out_acc[:, h, ts], out_acc[:, h, ts], wt[:]
                            )
                    if e == E - 1:
                        # stream each finished h-slice out during the last expert
                        nc.sync.dma_start(oT[:, h, :], out_acc[:, h, :])
    nc.compile()
    return nc


def _get_nc():
    if "nc" not in _cache:
        _cache["nc"] = _build_nc()
    return _cache["nc"]


def _install_ntff_hook():
    """Register the antenv.axon_hooks NTFF profile hook if the image lacks it
    (used only when MOE_TRACE=1; normal runs never touch this)."""
    import contextlib
    import ctypes
    import sys
    import types

    try:
        import antenv.axon_hooks  # noqa: F401

        return
    except ImportError:
        pass
    try:
        mod = types.ModuleType("antenv.axon_hooks")
        state = {"hook": None}
        mod.set_axon_ntff_profile_hook = lambda h: state.__setitem__("hook", h)
        mod.get_axon_ntff_profile_hook = lambda: state["hook"]
        sys.modules["antenv.axon_hooks"] = mod
        import antenv

        antenv.axon_hooks = mod

        lib = ctypes.CDLL("/opt/axon/libaxon_pjrt.so")
        if not hasattr(lib, "axon_start_nrt_profile"):
            return
        lib.axon_start_nrt_profile.argtypes = [
            ctypes.POINTER(ctypes.c_int64),
            ctypes.c_size_t,
        ]
        lib.axon_start_nrt_profile.restype = ctypes.c_int64
        lib.axon_stop_nrt_profile.argtypes = [ctypes.c_char_p]
        lib.axon_stop_nrt_profile.restype = ctypes.c_int64

        @contextlib.contextmanager
        def _hook(output_dir, device_ids):
            import jax

            jax.devices()
            if device_ids:
                ids = (ctypes.c_int64 * len(device_ids))(*device_ids)
                rc = lib.axon_start_nrt_profile(ids, len(device_ids))
            else:
                rc = lib.axon_start_nrt_profile(None, 0)
            if rc != 0:
                raise RuntimeError(f"axon_start_nrt_profile rc={rc}")
            try:
                yield
            finally:
                lib.axon_stop_nrt_profile(str(output_dir).encode())

        mod.set_axon_ntff_profile_hook(_hook)
    except Exception:
        pass


def _prep_shared(gate_w, up_w, down_w):
    """Host-side swizzle of the stacked expert weights into the DMA-friendly
    pre-tiled layouts (same arrays shared by all cores)."""
    gw = np.ascontiguousarray(
        gate_w.reshape(E, NK, P, NI, P).transpose(0, 3, 2, 1, 4), dtype=np.float32
    )
    uw = np.ascontiguousarray(
        up_w.reshape(E, NK, P, NI, P).transpose(0, 3, 2, 1, 4), dtype=np.float32
    )
    dw = (
        down_w.reshape(E, NI, P, NH, P)
        .transpose(0, 3, 2, 1, 4)
        .astype(np.float16)
    )
    dw = np.ascontiguousarray(dw)
    return gw, uw, dw


def kernel(hidden_states, router_w, router_b, gate_w, up_w, down_w):
    from concourse.bass_utils import run_bass_kernel_spmd

    hidden_states = np.asarray(hidden_states, dtype=np.float32)
    router_w = np.asarray(router_w, dtype=np.float32)
    router_b = np.asarray(router_b, dtype=np.float32)
    gate_w = np.asarray(gate_w, dtype=np.float32)
    up_w = np.asarray(up_w, dtype=np.float32)
    down_w = np.asarray(down_w, dtype=np.float32)

    X = hidden_states.reshape(B * S, H)

    # ---- host routing (tiny: [8192,1024]x[1024,8]) + balance loss, fp32 ----
    logits = X @ router_w.T + router_b
    logits = logits - logits.max(axis=-1, keepdims=True)
    ex = np.exp(logits)
    rw = ex / ex.sum(axis=-1, keepdims=True)  # [B*S, E] fp32
    mean_routing = rw.reshape(B, S, E).mean(axis=0)
    balance_loss = np.float32(
        np.mean((mean_routing - np.float32(1.0 / E)) ** 2) * np.float32(BALANCE_COEF)
    )

    gw, uw, dw = _prep_shared(gate_w, up_w, down_w)

    in_maps = []
    for c in range(NCORES):
        tok = slice(c * TC, (c + 1) * TC)
        xr = np.ascontiguousarray(
            X[tok].T.reshape(NK, P, TC).transpose(1, 0, 2)
        )  # [P, NK, TC] fp32
        rw_c = rw[tok].T  # [E, TC]
        wb = np.ascontiguousarray(
            np.broadcast_to(rw_c[:, None, :], (E, P, TC)), dtype=np.float32
        )
        in_maps.append({"xr": xr, "gw": gw, "uw": uw, "dw": dw, "wb": wb})

    nc = _get_nc()
    import os

    trace = os.environ.get("MOE_TRACE", "0") == "1"
    if trace:
        _install_ntff_hook()
    res = run_bass_kernel_spmd(
        nc, in_maps, core_ids=list(range(NCORES)), trace=trace
    )
    _cache["last_result"] = res

    out = np.empty((B * S, H), dtype=np.float32)
    for c in range(NCORES):
        oT = res.results[c]["oT"]  # [P, NH, TC]
        out[c * TC : (c + 1) * TC] = oT.transpose(1, 0, 2).reshape(H, TC).T
    out = out.reshape(B, S, H)
    return out, balance_loss
